# revision 15
# baseline (speedup 1.0000x reference)
"""Bahdanau attention TRN2 kernel.

Full inputs -> shard batch over 8 NeuronCores (4 batches/core) -> bass/tile
kernel -> gather full outputs.

Per-core program (B_LOC=4, S=4096, D=U=512):
  for each local batch b:
    - DMA values[b] into SBUF as [128, 32, 512] fp32 (s = 128*i + p)
    - per s-tile i: PE-transpose the bf16 (bitcast high-half) view to get
      vT [d, s] chunks; proj = vT.T @ W1 (bf16 matmul, fp32 PSUM);
      x = proj + qp (DVE); t = tanh(x) (ACT); score = sum_u t*V (DVE fused
      mul+reduce)
    - softmax without max-shift (|score| <= ||V||_1 + |b| ~ 21, exp is safe
      in fp32); denominator via ones-matmul partition reduction
    - context = sum_i values_tile_i.T @ exp_col_i (bf16 matmul, PSUM
      accumulate), scaled by 1/denom at the end
"""

import sys

if "/opt/trn_rl_repo" not in sys.path:
    sys.path.insert(0, "/opt/trn_rl_repo")

import numpy as np
import ml_dtypes

B, S, D, U = 32, 4096, 512, 512
N_CORES = 8
B_LOC = B // N_CORES          # 4 batches per core
NT = S // 128                 # 32 s-tiles per batch
NC_CH = D // 128              # 4 contraction chunks

_compiled = None


def _build_program():
    import concourse.bass as bass
    import concourse.bacc as bacc
    import concourse.mybir as mybir
    import concourse.tile as tile

    f32 = mybir.dt.float32
    bf16 = mybir.dt.bfloat16
    AF = mybir.ActivationFunctionType
    ALU = mybir.AluOpType

    nc = bacc.Bacc("TRN2", target_bir_lowering=False, debug=False,
                   enable_asserts=True)

    vals = nc.dram_tensor("vals", [B_LOC, S, D], f32, kind="ExternalInput").ap()
    qT = nc.dram_tensor("qT", [128, NC_CH, B_LOC], bf16, kind="ExternalInput").ap()
    w1 = nc.dram_tensor("w1", [128, NC_CH, U], bf16, kind="ExternalInput").ap()
    w2 = nc.dram_tensor("w2", [128, NC_CH, U], bf16, kind="ExternalInput").ap()
    vb = nc.dram_tensor("vb", [128, U], bf16, kind="ExternalInput").ap()
    b1 = nc.dram_tensor("b1", [1, U], f32, kind="ExternalInput").ap()
    b2 = nc.dram_tensor("b2", [1, U], f32, kind="ExternalInput").ap()
    idb = nc.dram_tensor("idb", [128, 128], bf16, kind="ExternalInput").ap()

    attn = nc.dram_tensor("attn", [B_LOC, S], f32, kind="ExternalOutput").ap()
    ctx = nc.dram_tensor("ctx", [B_LOC, D], f32, kind="ExternalOutput").ap()

    # DRAM scratch for partition broadcasts (DMA replication from DRAM)
    qp_dram = nc.dram_tensor("qp_scratch", [B_LOC, U], f32).ap()
    bsum_dram = nc.dram_tensor("bsum_scratch", [1, U], f32).ap()

    from contextlib import ExitStack

    with tile.TileContext(nc) as tc, ExitStack() as es:
        # ---------------- pools ----------------
        const_p = es.enter_context(tc.tile_pool(name="const", bufs=1))
        vpool = es.enter_context(tc.tile_pool(name="vals", bufs=8))
        vt_sb_p = es.enter_context(tc.tile_pool(name="vtsb", bufs=4))
        x_p = es.enter_context(tc.tile_pool(name="x", bufs=3))
        t_p = es.enter_context(tc.tile_pool(name="t", bufs=3))
        junk_p = es.enter_context(tc.tile_pool(name="junk", bufs=1))
        sc_p = es.enter_context(tc.tile_pool(name="scores", bufs=2))
        ep_p = es.enter_context(tc.tile_pool(name="epi", bufs=2))
        qp_p = es.enter_context(tc.tile_pool(name="qp", bufs=B_LOC))
        ps_t = es.enter_context(tc.tile_pool(name="ps_t", bufs=2, space="PSUM"))
        ps_p = es.enter_context(tc.tile_pool(name="ps_p", bufs=3, space="PSUM"))
        ps_s = es.enter_context(tc.tile_pool(name="ps_s", bufs=1, space="PSUM"))

        # ---------------- constants / setup ----------------
        w1_sb = const_p.tile([128, NC_CH, U], bf16)
        nc.sync.dma_start(w1_sb[:], w1)
        w2_sb = const_p.tile([128, NC_CH, U], bf16)
        nc.sync.dma_start(w2_sb[:], w2)
        vb_sb = const_p.tile([128, U], bf16)
        nc.sync.dma_start(vb_sb[:], vb)
        qt_sb = const_p.tile([128, NC_CH, B_LOC], bf16)
        nc.sync.dma_start(qt_sb[:], qT)
        idb_sb = const_p.tile([128, 128], bf16)
        nc.sync.dma_start(idb_sb[:], idb)
        b1_sb = const_p.tile([1, U], f32)
        nc.sync.dma_start(b1_sb[:], b1)
        b2_sb = const_p.tile([1, U], f32)
        nc.sync.dma_start(b2_sb[:], b2)
        ones_col = const_p.tile([128, 1], f32)
        nc.vector.memset(ones_col[:], 1.0)
        ones_row = const_p.tile([1, 128], f32)
        nc.vector.memset(ones_row[:], 1.0)

        # bsum = b1 + b2 -> DRAM -> broadcast to [B_LOC, U]
        bsum_sb = const_p.tile([1, U], f32)
        nc.vector.tensor_add(bsum_sb[:], b1_sb[:], b2_sb[:])
        nc.sync.dma_start(bsum_dram, bsum_sb[:])
        bsum4 = const_p.tile([B_LOC, U], f32)
        nc.gpsimd.dma_start(out=bsum4[:], in_=bsum_dram.to_broadcast([B_LOC, U]))

        # qp_all[b, u] = q[b] @ W2 + b1 + b2
        qp_psum = ps_s.tile([B_LOC, U], f32, tag="ctxqp")
        for c in range(NC_CH):
            nc.tensor.matmul(qp_psum[:], qt_sb[:, c, :], w2_sb[:, c, :],
                             start=(c == 0), stop=(c == NC_CH - 1))
        qp_all = const_p.tile([B_LOC, U], f32)
        nc.vector.tensor_add(qp_all[:], qp_psum[:], bsum4[:])
        nc.sync.dma_start(qp_dram, qp_all[:])
        qp_bc = []
        for b in range(B_LOC):
            t = qp_p.tile([128, U], f32, tag=f"qpb{b}")
            nc.gpsimd.dma_start(
                out=t[:], in_=qp_dram[b : b + 1, :].to_broadcast([128, U]))
            qp_bc.append(t)

        # ---------------- main flat pipeline over all batches ----------------
        # Global software pipeline: stage lags keep each in-order engine fed
        # with an independent instruction while a dependent one waits.
        # exp(score) needs no max shift, so context matmuls run per-chunk
        # inside the batch instead of after the full softmax.
        NCH = 4                    # DMA chunks per batch (2MB each)
        TPC = NT // NCH            # s-tiles per chunk (8)
        vtiles = {}
        scores_t, e128_t, ebf_t, rs4_t, ctxp_t = {}, {}, {}, {}, {}
        vt_sbs, t_pairs, x_pairs = {}, {}, {}

        def emit_loads(b):
            src_b = vals[b].rearrange("(p i) d -> p i d", i=NT)
            tl = []
            for j in range(NCH):
                vt = vpool.tile([128, TPC, D], f32, tag="v",
                                name=f"v_{b}_{j}")
                nc.sync.dma_start(vt[:], src_b[:, TPC * j : TPC * (j + 1), :])
                tl.append(vt)
            vtiles[b] = tl

        def vbf_view(b, i):
            return vtiles[b][i // TPC].bitcast(bf16).rearrange(
                "p i (d two) -> p i d two", two=2)[:, i % TPC, :, 1]

        def stage_T(b, i):
            if i == 0:
                scores_t[b] = sc_p.tile([128, NT], f32, tag="sc",
                                        name=f"sc_{b}")
                e128_t[b] = ep_p.tile([128, NT], f32, tag="e128",
                                      name=f"e128_{b}")
                ebf_t[b] = ep_p.tile([128, NT], bf16, tag="e_bf",
                                     name=f"ebf_{b}")
                rs4_t[b] = ep_p.tile([128, NCH], f32, tag="rs4",
                                     name=f"rs4_{b}")
                ctxp_t[b] = ps_s.tile([1, D], f32, tag="ctxqp",
                                      name=f"ctxp_{b}")
            vt_psum = ps_t.tile([128, D], bf16, tag="vtp", name=f"vtp_{b}_{i}")
            vsrc = vbf_view(b, i)
            for c in range(NC_CH):
                nc.tensor.matmul(
                    vt_psum[:, 128 * c : 128 * (c + 1)],
                    vsrc[:, 128 * c : 128 * (c + 1)],
                    idb_sb[:], is_transpose=True,
                    start=(c == 0), stop=(c == NC_CH - 1))
            vt_sb = vt_sb_p.tile([128, D], bf16, tag="vts",
                                 name=f"vts_{b}_{i}")
            nc.scalar.copy(vt_sb[:], vt_psum[:])
            vt_sbs[(b, i)] = vt_sb

        def stage_P(b, i):
            vt_sb = vt_sbs.pop((b, i))
            proj = ps_p.tile([128, U], f32, tag="proj", name=f"proj_{b}_{i}")
            for c in range(NC_CH):
                nc.tensor.matmul(proj[:], vt_sb[:, 128 * c : 128 * (c + 1)],
                                 w1_sb[:, c, :],
                                 start=(c == 0), stop=(c == NC_CH - 1))
            if i % 2 == 0:
                x_pairs[b] = x_p.tile([128, 2, U], bf16, tag="x",
                                      name=f"x_{b}_{i}")
            x_pair = x_pairs[b]
            nc.vector.tensor_add(x_pair[:, i % 2, :], proj[:], qp_bc[b][:])
            if i % 2 == 1:
                t_pair = t_p.tile([128, 2, U], bf16, tag="t",
                                  name=f"t_{b}_{i}")
                nc.scalar.activation(t_pair[:], x_pair[:], AF.Tanh)
                t_pairs[(b, i - 1)] = t_pair
                t_pairs[(b, i)] = t_pair

        def stage_S(b, i):
            t_pair = t_pairs.pop((b, i))
            junk = junk_p.tile([128, U], bf16, tag="junk", name=f"junk_{b}_{i}")
            nc.vector.scalar_tensor_tensor(
                out=junk[:], in0=t_pair[:, i % 2, :], scalar=1.0,
                in1=vb_sb[:], op0=ALU.mult, op1=ALU.mult,
                accum_out=scores_t[b][:, i : i + 1])

        def stage_E(b, j):
            # exp + bf16 copy for chunk j (tiles 8j..8j+7)
            sl = slice(TPC * j, TPC * (j + 1))
            nc.scalar.activation(e128_t[b][:, sl], scores_t[b][:, sl], AF.Exp,
                                 accum_out=rs4_t[b][:, j : j + 1])
            nc.scalar.copy(ebf_t[b][:, sl], e128_t[b][:, sl])

        def stage_C(b, j):
            # context matmuls for chunk j
            ctx_psum = ctxp_t[b]
            for i in range(TPC * j, TPC * (j + 1)):
                nc.tensor.matmul(ctx_psum[:], ebf_t[b][:, i : i + 1],
                                 vbf_view(b, i),
                                 start=(i == 0), stop=(i == NT - 1))

        def emit_epilogue(b):
            scores_t.pop(b)
            e128 = e128_t.pop(b)
            ebf_t.pop(b)
            rs4 = rs4_t.pop(b)
            rowsum = ep_p.tile([128, 1], f32, tag="rowsum", name=f"rs_{b}")
            nc.vector.tensor_reduce(rowsum[:], rs4[:], mybir.AxisListType.X,
                                    ALU.add)
            denom = ps_s.tile([1, 1], f32, tag="small", name=f"dn_{b}")
            nc.tensor.matmul(denom[:], rowsum[:], ones_col[:])
            inv_sb = ep_p.tile([1, 1], f32, tag="inv", name=f"inv_{b}")
            nc.vector.reciprocal(inv_sb[:], denom[:])
            invb = ps_s.tile([128, 1], f32, tag="small", name=f"invb_{b}")
            nc.tensor.matmul(invb[:], ones_row[:], inv_sb[:])
            w128 = ep_p.tile([128, NT], f32, tag="w128", name=f"w_{b}")
            nc.vector.tensor_scalar_mul(w128[:], e128[:], invb[:])
            nc.sync.dma_start(
                attn[b : b + 1, :].rearrange("one (p i) -> (one p) i", i=NT),
                w128[:])
            ctx_psum = ctxp_t.pop(b)
            ctx_sb = ep_p.tile([1, D], f32, tag="ctx_sb", name=f"ctxs_{b}")
            nc.scalar.mul(ctx_sb[:], ctx_psum[:], inv_sb[:])
            nc.sync.dma_start(ctx[b : b + 1, :], ctx_sb[:])

        GT = B_LOC * NT
        emit_loads(0)
        for g in range(GT + 4):
            if g < GT:
                b_t, i_t = divmod(g, NT)
                if i_t == 0 and b_t + 1 < B_LOC:
                    emit_loads(b_t + 1)
                stage_T(b_t, i_t)
            if 1 <= g <= GT:
                stage_P(*divmod(g - 1, NT))
            if 2 <= g <= GT + 1:
                b_s, i_s = divmod(g - 2, NT)
                stage_S(b_s, i_s)
                if i_s % TPC == TPC - 1:
                    stage_E(b_s, i_s // TPC)
            if 4 <= g <= GT + 3:
                b_c, i_c = divmod(g - 4, NT)
                if i_c % TPC == TPC - 1:
                    stage_C(b_c, i_c // TPC)
                    if i_c == NT - 1:
                        emit_epilogue(b_c)

    nc.compile()
    return nc


def _get_program():
    global _compiled
    if _compiled is None:
        _compiled = _build_program()
    return _compiled


def kernel(query, values, W1, b1, W2, b2, V, bv):
    from concourse.bass_utils import run_bass_kernel_spmd

    query = np.asarray(query, dtype=np.float32)
    values = np.asarray(values, dtype=np.float32)
    W1 = np.asarray(W1, dtype=np.float32)
    b1 = np.asarray(b1, dtype=np.float32)
    W2 = np.asarray(W2, dtype=np.float32)
    b2 = np.asarray(b2, dtype=np.float32)
    V = np.asarray(V, dtype=np.float32)
    bv = np.asarray(bv, dtype=np.float32)

    bf = ml_dtypes.bfloat16
    # weight/layout prep (tiny, replicated across cores)
    w1b = np.ascontiguousarray(
        W1.reshape(NC_CH, 128, U).transpose(1, 0, 2)).astype(bf)
    w2b = np.ascontiguousarray(
        W2.reshape(NC_CH, 128, U).transpose(1, 0, 2)).astype(bf)
    vbb = np.ascontiguousarray(
        np.broadcast_to(V.reshape(1, U), (128, U))).astype(bf)
    b1r = b1.reshape(1, U)
    b2r = b2.reshape(1, U)
    idb = np.eye(128, dtype=np.float32).astype(bf)

    in_maps = []
    for c in range(N_CORES):
        qq = query[B_LOC * c : B_LOC * (c + 1)]          # [4, 512]
        qtb = np.ascontiguousarray(
            qq.T.reshape(NC_CH, 128, B_LOC).transpose(1, 0, 2)).astype(bf)
        in_maps.append({
            "vals": np.ascontiguousarray(values[B_LOC * c : B_LOC * (c + 1)]),
            "qT": qtb,
            "w1": w1b,
            "w2": w2b,
            "vb": vbb,
            "b1": b1r,
            "b2": b2r,
            "idb": idb,
        })

    nc = _get_program()
    res = run_bass_kernel_spmd(nc, in_maps, list(range(N_CORES)))

    context = np.empty((B, D), dtype=np.float32)
    attention_weights = np.empty((B, S, 1), dtype=np.float32)
    for c in range(N_CORES):
        context[B_LOC * c : B_LOC * (c + 1)] = res.results[c]["ctx"]
        attention_weights[B_LOC * c : B_LOC * (c + 1), :, 0] = \
            res.results[c]["attn"]
    return (context, attention_weights)
